# revision 16
# baseline (speedup 1.0000x reference)
"""GAT attention head (nn_AttHead_11330123727477) on 8 Trainium2 NeuronCores.

Reference computation:
    h = input @ W;  e_ij = leakyrelu(f_src_i + f_dst_j, 0.2)
    h' = elu(softmax_j(where(adj, e, -inf)) @ h)

Algebraic core (exact): with u'_i = exp(-0.8 f_src_i), v_j = exp(0.8 f_dst_j),
q_j = exp(0.2 f_dst_j), after dividing softmax row i by exp(0.8 f_src_i):
    att_ij ∝ A_ij * q_j * max(u'_i, v_j)

HYBRID form mix (balances all four engines; scores laid [j part, i free]):
  - D-chunks (even t, mask shipped as bf16 {0,1}):
        g = max(u'_bc, v_t)   (VectorE tensor_scalar, 4x)
        p = g * mask          (VectorE tensor_tensor, 2x, batched)
        psN += [q h | q]^T @ p            (full score, 2 matmuls)
  - A8-chunks (odd t, plane shipped as fp8: Y = where(A^T, fp8(u'_i), -240)):
        P2 = relu(Y - v_t)    (ScalarE activation, fp8 input, masked relu:
                               A=0 cells give relu(-240-v) = 0 exactly)
        psN += [q h | q]^T @ P2           (relu part)
        psM += fp8[q v h | q v]^T @ Y     (fp8 matmul; v-part recovered via
            Σ_{j∈A} qvh = (psM + B*C) / (u'_i + B),  B=240, C=Σ_j qvh)
    The B*C offset is folded into psM's group by a K=1 fp32 matmul at start;
    1/(u'+B) is a host fp32 row broadcast through a ones-matmul.
Epilogue: R = psN + psM*rcp_bc; h' = R[:64]/R[64]; elu; store.

The mix halves the 2-byte mask traffic (fp8 for half the chunks), moves half
the elementwise O(N^2) work to the otherwise-idle ScalarE (its rate is dtype
independent so it reads fp8 at full speed), and keeps PE at 3 matmuls/chunk
average. Engine model per core: PE ~41us, DVE ~34us, ScalarE ~37us, DMA ~41us.

Sharding: row-parallel over the 8192 output rows, 1024 rows per core,
no cross-core communication.
"""

import numpy as np
import ml_dtypes

N = 8192
IN_F = 128
OUT_F = 64
HT_F = OUT_F + 1  # [q*h | q] carries the denominator column
N_CORES = 8
SLAB = N // N_CORES  # 1024 output rows per core
P = 128
NT = N // P  # 64 j-chunks of 128
# chunk class split: D-chunks (VectorE form) vs A8-chunks (fp8/ScalarE form).
# 36:28 balances PE (~40us) vs DVE (~39us) vs ScalarE (~32us).
_D_EXTRA = (1, 17, 33, 49)
IS_D = [t % 2 == 0 or t in _D_EXTRA for t in range(64)]
D_LIST = [t for t in range(64) if IS_D[t]]
A_LIST = [t for t in range(64) if not IS_D[t]]
ND = len(D_LIST)  # 36
NA = len(A_LIST)  # 28
HALF = SLAB // 2  # PSUM free-dim limit for fp32 is 512
BIG = 240.0

_bf16 = ml_dtypes.bfloat16
_f8 = ml_dtypes.float8_e4m3  # TRN FP8_EXP4 (max +-240)

_nc_cache = None


def _build_bass():
    import concourse.mybir as mybir
    import concourse.tile as tile
    from concourse import bacc

    bf = mybir.dt.bfloat16
    f8 = mybir.dt.float8e4
    f32 = mybir.dt.float32
    Alu = mybir.AluOpType

    nc = bacc.Bacc("TRN2", target_bir_lowering=False, debug=False)

    maskD = nc.dram_tensor("maskD", [P, ND * SLAB], bf, kind="ExternalInput")
    y8 = nc.dram_tensor("y8", [P, NA * SLAB], f8, kind="ExternalInput")
    u_bc = nc.dram_tensor("u_bc", [P, SLAB], bf, kind="ExternalInput")
    vT = nc.dram_tensor("vT", [P, NT], f32, kind="ExternalInput")
    vTn = nc.dram_tensor("vTn", [P, NT], f32, kind="ExternalInput")
    qh = nc.dram_tensor("qh", [P, NT * HT_F], bf, kind="ExternalInput")
    qvh8 = nc.dram_tensor("qvh8", [P, NA * HT_F], f8, kind="ExternalInput")
    rcphl = nc.dram_tensor("rcphl", [2, SLAB], bf, kind="ExternalInput")
    bchl = nc.dram_tensor("bchl", [2, HT_F], bf, kind="ExternalInput")
    out = nc.dram_tensor("out", [OUT_F, SLAB], f32, kind="ExternalOutput")

    maskD_k = maskD.rearrange("p (k i) -> p k i", i=SLAB)
    y8_k = y8.rearrange("p (k i) -> p k i", i=SLAB)

    with tile.TileContext(nc) as tc:
        with (
            tc.tile_pool(name="const", bufs=1) as cpool,
            tc.tile_pool(name="mD", bufs=6) as mpool,
            tc.tile_pool(name="m8", bufs=6) as apool,
            tc.tile_pool(name="gt", bufs=4) as gpool,
            tc.tile_pool(name="p8", bufs=4) as ppool,
            tc.tile_pool(name="ps", bufs=1, space="PSUM") as pspool,
            tc.tile_pool(name="epi", bufs=1) as epool,
        ):
            vT_sb = cpool.tile([P, NT], f32)
            nc.sync.dma_start(vT_sb[:], vT[:])
            u_sb = cpool.tile([P, SLAB], bf)
            nc.sync.dma_start(u_sb[:], u_bc[:])
            rcp_sb = cpool.tile([2, SLAB], bf)
            nc.scalar.dma_start(rcp_sb[:], rcphl[:])
            bc_sb = cpool.tile([2, HT_F], bf)
            nc.scalar.dma_start(bc_sb[:], bchl[:])
            vTn_sb = cpool.tile([P, NT], f32)
            nc.scalar.dma_start(vTn_sb[:], vTn[:])
            # qh/qvh8 DMAs are deferred into the loop so the first plane
            # groups hit the wire immediately (they are needed only by the
            # first psN/psM matmuls, ~10us later)
            qh_sb = cpool.tile([P, NT, HT_F], bf)
            qvh8_sb = cpool.tile([P, NA, HT_F], f8)

            # Warm the ACT exp table (Relu lives in every set; no later switch)
            warm = cpool.tile([P, 8], f32)
            nc.scalar.activation(
                warm[:], vT_sb[:, 0:8], mybir.ActivationFunctionType.Exp
            )

            ones65 = cpool.tile([2, HT_F], bf)
            nc.vector.memset(ones65[:], 1.0)
            onesrow = cpool.tile([2, HALF], bf)
            nc.vector.memset(onesrow[:], 1.0)
            ones64 = cpool.tile([2, OUT_F], bf)
            nc.vector.memset(ones64[:], 1.0)

            # PSUM accumulators (one bank each, 8 banks total)
            psN0 = pspool.tile([HT_F, HALF], f32)
            psN1 = pspool.tile([HT_F, HALF], f32)
            psM0 = pspool.tile([HT_F, HALF], f32)
            psM1 = pspool.tile([HT_F, HALF], f32)
            pb0 = pspool.tile([HT_F, HALF], f32)
            pb1 = pspool.tile([HT_F, HALF], f32)
            pd0 = pspool.tile([OUT_F, HALF], f32)
            pd1 = pspool.tile([OUT_F, HALF], f32)

            # rcp broadcast across the 65 output partitions: K=2 matmul sums
            # the bf16 hi+lo rows in one 1-pass matmul per half
            nc.tensor.matmul(pb0[:], ones65[:], rcp_sb[:, 0:HALF])
            nc.tensor.matmul(pb1[:], ones65[:], rcp_sb[:, HALF:SLAB])
            pbS = epool.tile([HT_F, SLAB], f32)
            nc.vector.tensor_copy(out=pbS[:, 0:HALF], in_=pb0[:])
            nc.vector.tensor_copy(out=pbS[:, HALF:SLAB], in_=pb1[:])

            # init psM accumulation with the +B*C offset (K=2 bf16 hi/lo)
            nc.tensor.matmul(psM0[:], bc_sb[:], onesrow[:], start=True, stop=False)
            nc.tensor.matmul(psM1[:], bc_sb[:], onesrow[:], start=True, stop=False)

            # ---- main loop: groups of chunks (even t = D-form, odd = A8) ----
            groups = [(0, 2), (2, 2)] + [(4 + 6 * k, 6) for k in range(10)]
            n_issued_N = 0
            kD0 = 0
            kA0 = 0
            for gi, (t0g, grp) in enumerate(groups):
                if gi == 2:
                    nc.sync.dma_start(
                        qh_sb[:], qh.rearrange("p (t f) -> p t f", f=HT_F)
                    )
                    nc.scalar.dma_start(
                        qvh8_sb[:], qvh8.rearrange("p (k f) -> p k f", f=HT_F)
                    )
                Ds = [t for t in range(t0g, t0g + grp) if IS_D[t]]
                As = [t for t in range(t0g, t0g + grp) if not IS_D[t]]
                nD, nA = len(Ds), len(As)
                mD = mpool.tile([P, 4, SLAB], bf, tag="mD")
                nc.sync.dma_start(mD[:, 0:nD, :], maskD_k[:, kD0 : kD0 + nD, :])
                if nA:
                    m8 = apool.tile([P, 3, SLAB], f8, tag="m8")
                    nc.sync.dma_start(
                        m8[:, 0:nA, :], y8_k[:, kA0 : kA0 + nA, :]
                    )

                gt = gpool.tile([P, 4, SLAB], bf, tag="gt")
                p8t = ppool.tile([P, 3, SLAB], bf, tag="p8t")
                for b, tD in enumerate(Ds):
                    nc.vector.tensor_scalar(
                        gt[:, b, :], u_sb[:], vT_sb[:, tD : tD + 1], None, Alu.max
                    )
                for b, tA in enumerate(As):
                    nc.scalar.activation(
                        out=p8t[:, b, :],
                        in_=m8[:, b, :],
                        func=mybir.ActivationFunctionType.Relu,
                        bias=vTn_sb[:, tA : tA + 1],
                        scale=1.0,
                    )
                # batched mask multiply for the D-chunks
                nc.vector.tensor_tensor(
                    gt[:, 0:nD, :], gt[:, 0:nD, :], mD[:, 0:nD, :], Alu.mult
                )

                # PE: A8 psM first (depends only on the y8 DMA), then D psN,
                # then A8 psN (depends on ScalarE)
                for b in range(nA):
                    k = kA0 + b
                    lastM = k == NA - 1
                    nc.tensor.matmul(
                        psM0[:], qvh8_sb[:, k, :], m8[:, b, 0:HALF],
                        start=False, stop=lastM,
                    )
                    nc.tensor.matmul(
                        psM1[:], qvh8_sb[:, k, :], m8[:, b, HALF:SLAB],
                        start=False, stop=lastM,
                    )
                for b, tD in enumerate(Ds):
                    first = n_issued_N == 0
                    n_issued_N += 1
                    nc.tensor.matmul(
                        psN0[:], qh_sb[:, tD, :], gt[:, b, 0:HALF],
                        start=first, stop=False,
                    )
                    nc.tensor.matmul(
                        psN1[:], qh_sb[:, tD, :], gt[:, b, HALF:SLAB],
                        start=first, stop=False,
                    )
                for b, tA in enumerate(As):
                    n_issued_N += 1
                    lastN = n_issued_N == NT
                    nc.tensor.matmul(
                        psN0[:], qh_sb[:, tA, :], p8t[:, b, 0:HALF],
                        start=False, stop=lastN,
                    )
                    nc.tensor.matmul(
                        psN1[:], qh_sb[:, tA, :], p8t[:, b, HALF:SLAB],
                        start=False, stop=lastN,
                    )
                kD0 += nD
                kA0 += nA

            # ---- epilogue ----
            # R = psN + psM * rcp_bc   (psM already carries +B*C)
            t1 = epool.tile([HT_F, SLAB], f32)
            R = epool.tile([HT_F, SLAB], f32)
            nc.vector.tensor_tensor(t1[:, 0:HALF], pbS[:, 0:HALF], psM0[:], Alu.mult)
            nc.vector.tensor_tensor(
                t1[:, HALF:SLAB], pbS[:, HALF:SLAB], psM1[:], Alu.mult
            )
            nc.vector.tensor_tensor(R[:, 0:HALF], t1[:, 0:HALF], psN0[:], Alu.add)
            nc.vector.tensor_tensor(
                R[:, HALF:SLAB], t1[:, HALF:SLAB], psN1[:], Alu.add
            )

            # denominator reciprocal: spread the row over 128 partitions
            # (scalar queue: idle at this point, unlike sync)
            den128 = epool.tile([P, SLAB // P], f32)
            nc.scalar.dma_start(den128[:], R[OUT_F : OUT_F + 1, :])
            rcp128 = epool.tile([P, SLAB // P], f32)
            nc.vector.reciprocal(out=rcp128[:], in_=den128[:])
            # bf16 hi+lo split while spread; pd broadcast = one K=2 bf16
            # matmul per half
            hi128 = epool.tile([P, SLAB // P], bf)
            nc.vector.tensor_copy(out=hi128[:], in_=rcp128[:])
            lo128 = epool.tile([P, SLAB // P], bf)
            nc.vector.tensor_tensor(lo128[:], rcp128[:], hi128[:], Alu.subtract)
            rcpd2 = epool.tile([2, SLAB], bf)
            nc.scalar.dma_start(rcpd2[0:1, :], hi128[:])
            nc.scalar.dma_start(rcpd2[1:2, :], lo128[:])

            nc.tensor.matmul(pd0[:], ones64[:], rcpd2[:, 0:HALF])
            nc.tensor.matmul(pd1[:], ones64[:], rcpd2[:, HALF:SLAB])

            div = epool.tile([OUT_F, SLAB], f32)
            nc.vector.tensor_tensor(
                div[:, 0:HALF], R[0:OUT_F, 0:HALF], pd0[:], Alu.mult
            )
            nc.vector.tensor_tensor(
                div[:, HALF:SLAB], R[0:OUT_F, HALF:SLAB], pd1[:], Alu.mult
            )

            # elu(x) = relu(x) + min(exp(x) - 1, 0)  (bf16 intermediates)
            ex = epool.tile([OUT_F, SLAB], bf)
            nc.scalar.activation(ex[:], div[:], mybir.ActivationFunctionType.Exp)
            exm = epool.tile([OUT_F, SLAB], bf)
            nc.vector.tensor_scalar(exm[:], ex[:], 1.0, 0.0, Alu.subtract, Alu.min)
            rl = epool.tile([OUT_F, SLAB], bf)
            nc.vector.tensor_scalar(rl[:], div[:], 0.0, None, Alu.max)
            ov = epool.tile([OUT_F, SLAB], f32)
            nc.vector.tensor_tensor(ov[:], exm[:], rl[:], Alu.add)

            nc.sync.dma_start(out[:], ov[:])

    nc.finalize()
    return nc


def _get_nc():
    global _nc_cache
    if _nc_cache is None:
        _nc_cache = _build_bass()
    return _nc_cache


def prepare_inputs(input, adj, W, a):
    """Host-side O(N*F) precompute + input marshaling (elementwise mask remap
    only). Returns per-core input maps for the SPMD bass kernel."""
    f32 = np.float32
    input = np.asarray(input, dtype=f32)
    W = np.asarray(W, dtype=f32)
    a = np.asarray(a, dtype=f32)
    adj = np.asarray(adj)

    h = input @ W  # [N, 64]
    f_src = h @ a[:OUT_F]
    f_dst = h @ a[OUT_F:]

    u_b = np.exp(-0.8 * f_src).astype(_bf16)  # bf16 u' (D-chunks)
    u_f = u_b.astype(f32)
    u8 = u_f.astype(_f8)  # fp8 u' (A8-chunk plane values)
    u8_f = u8.astype(f32)
    v = np.exp(0.8 * f_dst).astype(f32)
    q = np.exp(0.2 * f_dst).astype(f32)

    htil = np.empty((N, HT_F), f32)
    htil[:, :OUT_F] = h * q[:, None]
    htil[:, OUT_F] = q
    qh_full = htil.astype(_bf16)
    hv8 = np.clip(htil * v[:, None], -BIG, BIG).astype(_f8)

    def dev_layout(x, take=None):
        y = x.reshape(NT, P, HT_F)
        if take is not None:
            y = y[take]
        k = y.shape[0]
        return np.ascontiguousarray(y.transpose(1, 0, 2).reshape(P, k * HT_F))

    qh_dev = dev_layout(qh_full)
    qvh8_dev = dev_layout(hv8, take=A_LIST)  # A8 chunks only

    # C over the A8 (odd) chunks, from the fp8-rounded stationary values
    C = hv8.astype(np.float64).reshape(NT, P, HT_F)[A_LIST].sum(axis=(0, 1))
    bc_f = (BIG * C).astype(f32)
    bc_hi = bc_f.astype(_bf16)
    bc_lo = (bc_f - bc_hi.astype(f32)).astype(_bf16)
    bchl_dev = np.ascontiguousarray(np.stack([bc_hi, bc_lo]))

    vT_dev = np.ascontiguousarray(v.reshape(NT, P).T)  # [128, 64] f32
    vTn_dev = np.ascontiguousarray(-vT_dev)

    ONE_BITS = np.uint16(0x3F80)  # bf16 1.0
    NEG8 = np.asarray(-BIG, _f8).view(np.uint8)
    u8_bits = u8.view(np.uint8)

    adjT = np.ascontiguousarray(adj.T != 0)  # [N(j), N(i)]

    in_maps = []
    for c in range(N_CORES):
        sl = slice(c * SLAB, (c + 1) * SLAB)
        at = adjT[:, sl].reshape(NT, P, SLAB)
        # D-plane: {1.0, 0} bf16, even chunks, [p, k, i] layout
        mDc = (at[D_LIST].astype(np.uint16) * ONE_BITS).transpose(1, 0, 2)
        maskD_c = np.ascontiguousarray(mDc).reshape(P, ND * SLAB).view(_bf16)
        # A8-plane: where(A, fp8(u'_i), -240), A8 chunks
        y8c = np.where(at[A_LIST], u8_bits[None, None, sl], NEG8).transpose(1, 0, 2)
        y8_c = np.ascontiguousarray(y8c).reshape(P, NA * SLAB).view(_f8)
        rcp_f = (1.0 / (u8_f[sl] + BIG)).astype(f32)
        rcp_hi = rcp_f.astype(_bf16)
        rcp_lo = (rcp_f - rcp_hi.astype(f32)).astype(_bf16)
        rcphl_c = np.ascontiguousarray(np.stack([rcp_hi, rcp_lo]))
        in_maps.append(
            {
                "maskD": maskD_c,
                "y8": y8_c,
                "u_bc": np.ascontiguousarray(
                    np.broadcast_to(u_b[sl][None, :], (P, SLAB))
                ),
                "vT": vT_dev,
                "vTn": vTn_dev,
                "qh": qh_dev,
                "qvh8": qvh8_dev,
                "rcphl": rcphl_c,
                "bchl": bchl_dev,
            }
        )
    return in_maps


def assemble_output(results):
    """results: list of 8 dicts with 'out' [64, 1024] f32 -> [N, 64] f32."""
    hp = np.empty((N, OUT_F), np.float32)
    for c in range(N_CORES):
        hp[c * SLAB : (c + 1) * SLAB] = results[c]["out"].T
    return hp


def kernel(input, adj, W, a):
    import time

    from concourse.bass_utils import run_bass_kernel_spmd

    nc = _get_nc()
    in_maps = prepare_inputs(input, adj, W, a)
    last_err = None
    for attempt in range(3):
        try:
            res = run_bass_kernel_spmd(nc, in_maps, core_ids=list(range(N_CORES)))
            return assemble_output(res.results)
        except Exception as e:  # transient device wedges have been observed
            last_err = e
            time.sleep(5)
    raise last_err


# revision 17
# speedup vs baseline: 1.0493x; 1.0493x over previous
"""GAT attention head (nn_AttHead_11330123727477) on 8 Trainium2 NeuronCores.

Reference computation:
    h = input @ W;  e_ij = leakyrelu(f_src_i + f_dst_j, 0.2)
    h' = elu(softmax_j(where(adj, e, -inf)) @ h)

Algebraic core (exact): with u'_i = exp(-0.8 f_src_i), v_j = exp(0.8 f_dst_j),
q_j = exp(0.2 f_dst_j), after dividing softmax row i by exp(0.8 f_src_i):
    att_ij ∝ A_ij * q_j * max(u'_i, v_j)

HYBRID form mix (balances all four engines; scores laid [j part, i free]):
  - D-chunks (even t, mask shipped as bf16 {0,1}):
        g = max(u'_bc, v_t)   (VectorE tensor_scalar, 4x)
        p = g * mask          (VectorE tensor_tensor, 2x, batched)
        psN += [q h | q]^T @ p            (full score, 2 matmuls)
  - A8-chunks (odd t, plane shipped as fp8: Y = where(A^T, fp8(u'_i), -240)):
        P2 = relu(Y - v_t)    (ScalarE activation, fp8 input, masked relu:
                               A=0 cells give relu(-240-v) = 0 exactly)
        psN += [q h | q]^T @ P2           (relu part)
        psM += fp8[q v h | q v]^T @ Y     (fp8 matmul; v-part recovered via
            Σ_{j∈A} qvh = (psM + B*C) / (u'_i + B),  B=240, C=Σ_j qvh)
    The B*C offset is folded into psM's group by a K=1 fp32 matmul at start;
    1/(u'+B) is a host fp32 row broadcast through a ones-matmul.
Epilogue: R = psN + psM*rcp_bc; h' = R[:64]/R[64]; elu; store.

The mix halves the 2-byte mask traffic (fp8 for half the chunks), moves half
the elementwise O(N^2) work to the otherwise-idle ScalarE (its rate is dtype
independent so it reads fp8 at full speed), and keeps PE at 3 matmuls/chunk
average. Engine model per core: PE ~41us, DVE ~34us, ScalarE ~37us, DMA ~41us.

Sharding: row-parallel over the 8192 output rows, 1024 rows per core,
no cross-core communication.
"""

import numpy as np
import ml_dtypes

N = 8192
IN_F = 128
OUT_F = 64
HT_F = OUT_F + 1  # [q*h | q] carries the denominator column
N_CORES = 8
SLAB = N // N_CORES  # 1024 output rows per core
P = 128
NT = N // P  # 64 j-chunks of 128
# chunk class split: D-chunks (VectorE form) vs A8-chunks (fp8/ScalarE form).
# 36:28 balances PE (~40us) vs DVE (~39us) vs ScalarE (~32us).
_D_EXTRA = (1, 17, 33, 49)
IS_D = [t % 2 == 0 or t in _D_EXTRA for t in range(64)]
D_LIST = [t for t in range(64) if IS_D[t]]
A_LIST = [t for t in range(64) if not IS_D[t]]
ND = len(D_LIST)  # 36
NA = len(A_LIST)  # 28
HALF = SLAB // 2  # PSUM free-dim limit for fp32 is 512
BIG = 240.0

_bf16 = ml_dtypes.bfloat16
_f8 = ml_dtypes.float8_e4m3  # TRN FP8_EXP4 (max +-240)

_nc_cache = None


def _build_bass():
    import concourse.mybir as mybir
    import concourse.tile as tile
    from concourse import bacc

    bf = mybir.dt.bfloat16
    f8 = mybir.dt.float8e4
    f32 = mybir.dt.float32
    Alu = mybir.AluOpType

    nc = bacc.Bacc("TRN2", target_bir_lowering=False, debug=False)

    maskD = nc.dram_tensor("maskD", [P, ND * SLAB], bf, kind="ExternalInput")
    y8 = nc.dram_tensor("y8", [P, NA * SLAB], f8, kind="ExternalInput")
    u_bc = nc.dram_tensor("u_bc", [P, SLAB], bf, kind="ExternalInput")
    vT = nc.dram_tensor("vT", [P, NT], f32, kind="ExternalInput")
    vTn = nc.dram_tensor("vTn", [P, NT], f32, kind="ExternalInput")
    qh = nc.dram_tensor("qh", [P, NT * HT_F], bf, kind="ExternalInput")
    qvh8 = nc.dram_tensor("qvh8", [P, NA * HT_F], f8, kind="ExternalInput")
    rcphl = nc.dram_tensor("rcphl", [2, SLAB], bf, kind="ExternalInput")
    bchl = nc.dram_tensor("bchl", [2, HT_F], bf, kind="ExternalInput")
    out = nc.dram_tensor("out", [OUT_F, SLAB], f32, kind="ExternalOutput")

    maskD_k = maskD.rearrange("p (k i) -> p k i", i=SLAB)
    y8_k = y8.rearrange("p (k i) -> p k i", i=SLAB)

    with tile.TileContext(nc) as tc:
        with (
            tc.tile_pool(name="const", bufs=1) as cpool,
            tc.tile_pool(name="mD", bufs=6) as mpool,
            tc.tile_pool(name="m8", bufs=6) as apool,
            tc.tile_pool(name="gt", bufs=4) as gpool,
            tc.tile_pool(name="p8", bufs=4) as ppool,
            tc.tile_pool(name="ps", bufs=1, space="PSUM") as pspool,
            tc.tile_pool(name="epi", bufs=1) as epool,
        ):
            vT_sb = cpool.tile([P, NT], f32)
            nc.sync.dma_start(vT_sb[:], vT[:])
            u_sb = cpool.tile([P, SLAB], bf)
            nc.sync.dma_start(u_sb[:], u_bc[:])
            rcp_sb = cpool.tile([2, SLAB], bf)
            nc.scalar.dma_start(rcp_sb[:], rcphl[:])
            bc_sb = cpool.tile([2, HT_F], bf)
            nc.scalar.dma_start(bc_sb[:], bchl[:])
            vTn_sb = cpool.tile([P, NT], f32)
            nc.scalar.dma_start(vTn_sb[:], vTn[:])
            # qh/qvh8 DMAs are deferred into the loop so the first plane
            # groups hit the wire immediately (they are needed only by the
            # first psN/psM matmuls, ~10us later)
            qh_sb = cpool.tile([P, NT, HT_F], bf)
            qvh8_sb = cpool.tile([P, NA, HT_F], f8)

            # Warm the natural_log_exp ACT set (Ln + Exp + Relu filler): the
            # epilogue uses Ln/Exp for the reciprocal and Exp for elu with no
            # table switch.
            warm = cpool.tile([P, 8], f32)
            nc.scalar.activation(
                warm[:], vT_sb[:, 0:8], mybir.ActivationFunctionType.Ln
            )

            ones65 = cpool.tile([2, HT_F], bf)
            nc.vector.memset(ones65[:], 1.0)
            onesrow = cpool.tile([2, HALF], bf)
            nc.vector.memset(onesrow[:], 1.0)
            ones64 = cpool.tile([2, OUT_F], bf)
            nc.vector.memset(ones64[:], 1.0)

            # PSUM accumulators (one bank each, 8 banks total)
            psN0 = pspool.tile([HT_F, HALF], f32)
            psN1 = pspool.tile([HT_F, HALF], f32)
            psM0 = pspool.tile([HT_F, HALF], f32)
            psM1 = pspool.tile([HT_F, HALF], f32)
            pb0 = pspool.tile([HT_F, HALF], f32)
            pb1 = pspool.tile([HT_F, HALF], f32)
            pd0 = pspool.tile([OUT_F, HALF], f32)
            pd1 = pspool.tile([OUT_F, HALF], f32)

            # rcp broadcast across the 65 output partitions: K=2 matmul sums
            # the bf16 hi+lo rows in one 1-pass matmul per half
            nc.tensor.matmul(pb0[:], ones65[:], rcp_sb[:, 0:HALF])
            nc.tensor.matmul(pb1[:], ones65[:], rcp_sb[:, HALF:SLAB])
            pbS = epool.tile([HT_F, SLAB], f32)
            nc.vector.tensor_copy(out=pbS[:, 0:HALF], in_=pb0[:])
            nc.vector.tensor_copy(out=pbS[:, HALF:SLAB], in_=pb1[:])

            # init psM accumulation with the +B*C offset (K=2 bf16 hi/lo)
            nc.tensor.matmul(psM0[:], bc_sb[:], onesrow[:], start=True, stop=False)
            nc.tensor.matmul(psM1[:], bc_sb[:], onesrow[:], start=True, stop=False)

            # ---- main loop: groups of chunks (even t = D-form, odd = A8) ----
            groups = [(0, 2), (2, 2)] + [(4 + 6 * k, 6) for k in range(10)]
            n_issued_N = 0
            kD0 = 0
            kA0 = 0
            for gi, (t0g, grp) in enumerate(groups):
                if gi == 2:
                    nc.sync.dma_start(
                        qh_sb[:], qh.rearrange("p (t f) -> p t f", f=HT_F)
                    )
                    nc.scalar.dma_start(
                        qvh8_sb[:], qvh8.rearrange("p (k f) -> p k f", f=HT_F)
                    )
                Ds = [t for t in range(t0g, t0g + grp) if IS_D[t]]
                As = [t for t in range(t0g, t0g + grp) if not IS_D[t]]
                nD, nA = len(Ds), len(As)
                mD = mpool.tile([P, 4, SLAB], bf, tag="mD")
                nc.sync.dma_start(mD[:, 0:nD, :], maskD_k[:, kD0 : kD0 + nD, :])
                if nA:
                    m8 = apool.tile([P, 3, SLAB], f8, tag="m8")
                    nc.sync.dma_start(
                        m8[:, 0:nA, :], y8_k[:, kA0 : kA0 + nA, :]
                    )

                gt = gpool.tile([P, 4, SLAB], bf, tag="gt")
                p8t = ppool.tile([P, 3, SLAB], bf, tag="p8t")
                for b, tD in enumerate(Ds):
                    nc.vector.tensor_scalar(
                        gt[:, b, :], u_sb[:], vT_sb[:, tD : tD + 1], None, Alu.max
                    )
                for b, tA in enumerate(As):
                    nc.scalar.activation(
                        out=p8t[:, b, :],
                        in_=m8[:, b, :],
                        func=mybir.ActivationFunctionType.Relu,
                        bias=vTn_sb[:, tA : tA + 1],
                        scale=1.0,
                    )
                # batched mask multiply for the D-chunks
                nc.vector.tensor_tensor(
                    gt[:, 0:nD, :], gt[:, 0:nD, :], mD[:, 0:nD, :], Alu.mult
                )

                # PE: A8 psM first (depends only on the y8 DMA), then D psN,
                # then A8 psN (depends on ScalarE)
                for b in range(nA):
                    k = kA0 + b
                    lastM = k == NA - 1
                    nc.tensor.matmul(
                        psM0[:], qvh8_sb[:, k, :], m8[:, b, 0:HALF],
                        start=False, stop=lastM,
                    )
                    nc.tensor.matmul(
                        psM1[:], qvh8_sb[:, k, :], m8[:, b, HALF:SLAB],
                        start=False, stop=lastM,
                    )
                for b, tD in enumerate(Ds):
                    first = n_issued_N == 0
                    n_issued_N += 1
                    nc.tensor.matmul(
                        psN0[:], qh_sb[:, tD, :], gt[:, b, 0:HALF],
                        start=first, stop=False,
                    )
                    nc.tensor.matmul(
                        psN1[:], qh_sb[:, tD, :], gt[:, b, HALF:SLAB],
                        start=first, stop=False,
                    )
                for b, tA in enumerate(As):
                    n_issued_N += 1
                    lastN = n_issued_N == NT
                    nc.tensor.matmul(
                        psN0[:], qh_sb[:, tA, :], p8t[:, b, 0:HALF],
                        start=False, stop=lastN,
                    )
                    nc.tensor.matmul(
                        psN1[:], qh_sb[:, tA, :], p8t[:, b, HALF:SLAB],
                        start=False, stop=lastN,
                    )
                kD0 += nD
                kA0 += nA

            # ---- epilogue ----
            # R = psN + psM * rcp_bc   (psM already carries +B*C)
            t1 = epool.tile([HT_F, SLAB], f32)
            R = epool.tile([HT_F, SLAB], f32)
            nc.vector.tensor_tensor(t1[:, 0:HALF], pbS[:, 0:HALF], psM0[:], Alu.mult)
            nc.vector.tensor_tensor(
                t1[:, HALF:SLAB], pbS[:, HALF:SLAB], psM1[:], Alu.mult
            )
            nc.vector.tensor_tensor(R[:, 0:HALF], t1[:, 0:HALF], psN0[:], Alu.add)
            nc.vector.tensor_tensor(
                R[:, HALF:SLAB], t1[:, HALF:SLAB], psN1[:], Alu.add
            )

            # denominator reciprocal = exp(-ln(den)) on ScalarE: no DMA
            # round-trip, both functions in the warmed table set. bf16 rcp
            # (2^-9 rel) is inside the error budget.
            lnd = epool.tile([1, SLAB], f32)
            nc.scalar.activation(
                lnd[:], R[OUT_F : OUT_F + 1, :], mybir.ActivationFunctionType.Ln
            )
            rcpb = epool.tile([1, SLAB], bf)
            nc.scalar.activation(
                rcpb[:], lnd[:], mybir.ActivationFunctionType.Exp, scale=-1.0
            )
            ones64s = cpool.tile([1, OUT_F], bf)
            nc.vector.memset(ones64s[:], 1.0)
            nc.tensor.matmul(pd0[:], ones64s[:], rcpb[:, 0:HALF])
            nc.tensor.matmul(pd1[:], ones64s[:], rcpb[:, HALF:SLAB])

            # per-half divide + elu + store: the first half's output DMA
            # overlaps the second half's epilogue compute
            div = epool.tile([OUT_F, SLAB], f32)
            ex = epool.tile([OUT_F, SLAB], bf)
            exm = epool.tile([OUT_F, SLAB], bf)
            rl = epool.tile([OUT_F, SLAB], bf)
            ov = epool.tile([OUT_F, SLAB], f32)
            for h, pd in ((0, pd0), (1, pd1)):
                HS = slice(h * HALF, (h + 1) * HALF)
                nc.vector.tensor_tensor(div[:, HS], R[0:OUT_F, HS], pd[:], Alu.mult)
                nc.scalar.activation(
                    ex[:, HS], div[:, HS], mybir.ActivationFunctionType.Exp
                )
                nc.vector.tensor_scalar(
                    exm[:, HS], ex[:, HS], 1.0, 0.0, Alu.subtract, Alu.min
                )
                nc.vector.tensor_scalar(rl[:, HS], div[:, HS], 0.0, None, Alu.max)
                nc.vector.tensor_tensor(ov[:, HS], exm[:, HS], rl[:, HS], Alu.add)
                nc.sync.dma_start(out[:, HS], ov[:, HS])

    nc.finalize()
    return nc


def _get_nc():
    global _nc_cache
    if _nc_cache is None:
        _nc_cache = _build_bass()
    return _nc_cache


def prepare_inputs(input, adj, W, a):
    """Host-side O(N*F) precompute + input marshaling (elementwise mask remap
    only). Returns per-core input maps for the SPMD bass kernel."""
    f32 = np.float32
    input = np.asarray(input, dtype=f32)
    W = np.asarray(W, dtype=f32)
    a = np.asarray(a, dtype=f32)
    adj = np.asarray(adj)

    h = input @ W  # [N, 64]
    f_src = h @ a[:OUT_F]
    f_dst = h @ a[OUT_F:]

    u_b = np.exp(-0.8 * f_src).astype(_bf16)  # bf16 u' (D-chunks)
    u_f = u_b.astype(f32)
    u8 = u_f.astype(_f8)  # fp8 u' (A8-chunk plane values)
    u8_f = u8.astype(f32)
    v = np.exp(0.8 * f_dst).astype(f32)
    q = np.exp(0.2 * f_dst).astype(f32)

    htil = np.empty((N, HT_F), f32)
    htil[:, :OUT_F] = h * q[:, None]
    htil[:, OUT_F] = q
    qh_full = htil.astype(_bf16)
    hv8 = np.clip(htil * v[:, None], -BIG, BIG).astype(_f8)

    def dev_layout(x, take=None):
        y = x.reshape(NT, P, HT_F)
        if take is not None:
            y = y[take]
        k = y.shape[0]
        return np.ascontiguousarray(y.transpose(1, 0, 2).reshape(P, k * HT_F))

    qh_dev = dev_layout(qh_full)
    qvh8_dev = dev_layout(hv8, take=A_LIST)  # A8 chunks only

    # C over the A8 (odd) chunks, from the fp8-rounded stationary values
    C = hv8.astype(np.float64).reshape(NT, P, HT_F)[A_LIST].sum(axis=(0, 1))
    bc_f = (BIG * C).astype(f32)
    bc_hi = bc_f.astype(_bf16)
    bc_lo = (bc_f - bc_hi.astype(f32)).astype(_bf16)
    bchl_dev = np.ascontiguousarray(np.stack([bc_hi, bc_lo]))

    vT_dev = np.ascontiguousarray(v.reshape(NT, P).T)  # [128, 64] f32
    vTn_dev = np.ascontiguousarray(-vT_dev)

    ONE_BITS = np.uint16(0x3F80)  # bf16 1.0
    NEG8 = np.asarray(-BIG, _f8).view(np.uint8)
    u8_bits = u8.view(np.uint8)

    adjT = np.ascontiguousarray(adj.T != 0)  # [N(j), N(i)]

    in_maps = []
    for c in range(N_CORES):
        sl = slice(c * SLAB, (c + 1) * SLAB)
        at = adjT[:, sl].reshape(NT, P, SLAB)
        # D-plane: {1.0, 0} bf16, even chunks, [p, k, i] layout
        mDc = (at[D_LIST].astype(np.uint16) * ONE_BITS).transpose(1, 0, 2)
        maskD_c = np.ascontiguousarray(mDc).reshape(P, ND * SLAB).view(_bf16)
        # A8-plane: where(A, fp8(u'_i), -240), A8 chunks
        y8c = np.where(at[A_LIST], u8_bits[None, None, sl], NEG8).transpose(1, 0, 2)
        y8_c = np.ascontiguousarray(y8c).reshape(P, NA * SLAB).view(_f8)
        rcp_f = (1.0 / (u8_f[sl] + BIG)).astype(f32)
        rcp_hi = rcp_f.astype(_bf16)
        rcp_lo = (rcp_f - rcp_hi.astype(f32)).astype(_bf16)
        rcphl_c = np.ascontiguousarray(np.stack([rcp_hi, rcp_lo]))
        in_maps.append(
            {
                "maskD": maskD_c,
                "y8": y8_c,
                "u_bc": np.ascontiguousarray(
                    np.broadcast_to(u_b[sl][None, :], (P, SLAB))
                ),
                "vT": vT_dev,
                "vTn": vTn_dev,
                "qh": qh_dev,
                "qvh8": qvh8_dev,
                "rcphl": rcphl_c,
                "bchl": bchl_dev,
            }
        )
    return in_maps


def assemble_output(results):
    """results: list of 8 dicts with 'out' [64, 1024] f32 -> [N, 64] f32."""
    hp = np.empty((N, OUT_F), np.float32)
    for c in range(N_CORES):
        hp[c * SLAB : (c + 1) * SLAB] = results[c]["out"].T
    return hp


def kernel(input, adj, W, a):
    import time

    from concourse.bass_utils import run_bass_kernel_spmd

    nc = _get_nc()
    in_maps = prepare_inputs(input, adj, W, a)
    last_err = None
    for attempt in range(3):
        try:
            res = run_bass_kernel_spmd(nc, in_maps, core_ids=list(range(N_CORES)))
            return assemble_output(res.results)
        except Exception as e:  # transient device wedges have been observed
            last_err = e
            time.sleep(5)
    raise last_err
